# revision 49
# baseline (speedup 1.0000x reference)
"""Trainium2 Bass kernel for nn_LinearReg_55508157333593.

Computes: loss = (c_omega * 0.001 / N) * sum over all rows/groups of
L2 norms of 25-element groups of weight [100000, 800] f32.

Since each row is 32 contiguous groups of 25 floats and rows are contiguous,
the whole buffer is just 3.2M consecutive 25-float groups. We shard the flat
array across 8 NeuronCores (10M floats each) and stream each core's slab
through SBUF as [128, 78125] (each partition owns 3125 consecutive groups).

Raw-Bass manual pipeline (no Tile, no Block barrier), per chunk i:
  SP:  DMA chunk i into input slot i%B       (per-slot completion sems;
       the first chunks go out over the ACT and gpsimd queues too, so
       several descriptor generators feed the SDMA engines during the ramp)
  ACT: square chunk i f32 -> bf16 sq ring
  DVE: pair-fold in place (cols 13:25 += cols 0:12, tensor_tensor runs
       2 elem/cycle for bf16) then reduce the 13 contiguous cols 12:25
       per 25-group -> f32 gs_all [128, 3125]. tensor_reduce is capped at
       1 elem/cycle for every dtype, so the fold cuts DVE ~20%/chunk and
       keeps the reduce pipeline ahead of the DMA stream even when HBM
       delivers ~390 GB/s (it varies 320-390 run to run).
The schedule ends with a run of small chunks so the final serial
square+reduce chain after the last DMA byte is short.

Endgame: ACT sqrts gs_all segments in place (bulk segments overlap the
stream), each with a fused per-partition row-sum (accum_out -> pr column),
then SP DMAs pr [128, n_segs] out. The host sums partitions/segments/cores
in float64 and applies the scaling (the "all-reduce the scalar" gather
step). A dummy Sqrt is ACT's first instruction so the activation tables
load before the stream needs ACT.

bf16 precision note: squares and pair-sums are quantized to 8 mantissa
bits (~0.4% relative) but group sums accumulate in f32 and the loss is a
mean over 3.2M groups, so quantization noise averages out (measured
~2e-5 rel err, tolerance 2e-2).
"""

import sys
import types

import numpy as np

if "/opt/trn_rl_repo" not in sys.path:
    sys.path.insert(0, "/opt/trn_rl_repo")


def _ensure_axon_ntff_hook():
    """Provide antenv.axon_hooks if the image's antenv lacks it.

    concourse.bass_utils imports it when BASS_TRACE is set under axon;
    without it the run crashes. If we install the shim we also register
    the ctypes-based NTFF hook the way trn_agent_boot.boot would have.
    """
    try:
        import antenv.axon_hooks  # noqa: F401
        return
    except ImportError:
        pass
    try:
        import antenv
    except ImportError:
        return
    mod = types.ModuleType("antenv.axon_hooks")
    mod._hook = None

    def set_axon_ntff_profile_hook(hook):
        mod._hook = hook

    def get_axon_ntff_profile_hook():
        return mod._hook

    mod.set_axon_ntff_profile_hook = set_axon_ntff_profile_hook
    mod.get_axon_ntff_profile_hook = get_axon_ntff_profile_hook
    sys.modules["antenv.axon_hooks"] = mod
    antenv.axon_hooks = mod
    try:
        from trn_agent_boot.trn_boot import _ntff_profile_via_ctypes

        hook = _ntff_profile_via_ctypes("/opt/axon/libaxon_pjrt.so")
        if hook is not None:
            set_axon_ntff_profile_hook(hook)
    except Exception:
        pass


_ensure_axon_ntff_hook()

N_CORES = 8
P = 128                      # SBUF partitions
GROUP = 25                   # elements per group
C_OMEGA = 0.001
N_ROWS = 100000
ROW = 800                    # elements per row
F_PER_PART = (N_ROWS * ROW) // (N_CORES * P)   # 78125 floats/partition/core

# chunk schedule (floats per partition; multiples of GROUP, sums to 78125):
# big chunks for streaming, then a long small-chunk tail so the last big
# reduce (the slowest compute op) finishes while small chunks still stream
# and the serial chain after the last DMA byte stays short.
SCHEDULE = [3125] * 23 + [625] * 8 + [500, 375, 250, 125]
# sqrt segments in chunk indices
SEG_BOUNDS = [6, 12, 18, 23, 35]
# seg index -> emit its sqrt just before this chunk's square piece(s).
# The whole tail is ONE segment emitted after the last square: ACT drains
# every remaining square uninterrupted (their dma sems complete early),
# DVE's reduces catch up in parallel, then a single sqrt+acc+DMA finishes —
# any seg sqrt mid-ladder serializes the tail's ACT<->DVE ping-pong.
SEG_EMIT_BEFORE = {0: 9, 1: 15, 2: 21, 3: 27}
# chunk -> number of column strips its DMA is split into, alternating between
# SP's and ACT's HWDGE queues. The SDMA engines come online instruction by
# instruction (~5 engines on the first, the rest ~3us later), so splitting the
# first chunks into many small instructions on two queues onboards all 16
# engines much sooner.
STRIPE_CHUNKS = {0: 2, 1: 2}
# chunks issued from gpsimd's SWDGE queue (a third descriptor generator and a
# separate ring): Pool is otherwise idle, and more queues engage more SDMA
# engines during the ramp.
GP_DMA_CHUNKS = (2, 3)
# alternate the remaining chunks between SP's and ACT's HWDGE queues.
# Measured neutral (stream rate and tail unchanged at equal HBM conditions),
# so it stays off; the code path is kept for future experiments.
ACT_ALTERNATE = False

_compiled = None
LAST_RESULTS = None          # BassKernelResults of the most recent run


def build(f_per_part=F_PER_PART, schedule=None, in_bufs=8, sq_bufs=8,
          seg_bounds=None, seg_emit_before=None, stripe_chunks=None,
          gp_dma_chunks=None, act_alternate=False):
    """Build and compile the per-core raw-Bass program."""
    from concourse import bacc, mybir

    if schedule is None:
        schedule = SCHEDULE
        seg_bounds = SEG_BOUNDS
        seg_emit_before = SEG_EMIT_BEFORE
        stripe_chunks = STRIPE_CHUNKS
        gp_dma_chunks = GP_DMA_CHUNKS
        act_alternate = ACT_ALTERNATE
    if stripe_chunks is None:
        stripe_chunks = {}
    if gp_dma_chunks is None:
        gp_dma_chunks = ()
    gp_dma_chunks = set(gp_dma_chunks)
    assert all(c < in_bufs for c in stripe_chunks), \
        "striped chunks must be first-use slots (no reuse wait on ACT)"
    assert all(c < in_bufs and c not in stripe_chunks for c in gp_dma_chunks)
    n = len(schedule)
    if seg_bounds is None:
        seg_bounds = [n]
    if seg_emit_before is None:
        seg_emit_before = {}
    assert sum(schedule) == f_per_part
    assert all(s % GROUP == 0 for s in schedule)
    assert seg_bounds[-1] == n and sorted(seg_bounds) == seg_bounds
    offs = [sum(schedule[:i]) for i in range(n)]
    gpcs = [s // GROUP for s in schedule]
    goffs = [sum(gpcs[:i]) for i in range(n + 1)]
    total_g = goffs[n]
    n_segs = len(seg_bounds)
    # (end_chunk, gstart, gend) per sqrt segment
    segs = []
    prev = 0
    for b in seg_bounds:
        segs.append((b, goffs[prev], goffs[b]))
        prev = b
    max_sz = max(schedule)
    f32 = mybir.dt.float32
    bf16 = mybir.dt.bfloat16
    Act = mybir.ActivationFunctionType

    nc = bacc.Bacc("TRN2", target_bir_lowering=False, debug=False,
                   num_devices=N_CORES)
    x = nc.dram_tensor("x", [P, f_per_part], f32, kind="ExternalInput").ap()
    out = nc.dram_tensor("out", [P, n_segs], f32, kind="ExternalOutput").ap()

    Bi = in_bufs
    Bs = sq_bufs
    in_ring = nc.alloc_sbuf_tensor("in_ring", [P, Bi * max_sz], f32).ap()
    sq_ring = nc.alloc_sbuf_tensor("sq_ring", [P, Bs * max_sz], bf16).ap()
    t_in = [in_ring[:, b * max_sz:(b + 1) * max_sz] for b in range(Bi)]
    t_sq = [sq_ring[:, b * max_sz:(b + 1) * max_sz] for b in range(Bs)]

    # one square+reduce PIECE per chunk, except the first two chunks are
    # split in half so DVE's pipeline wakes up earlier (its first wait is
    # released by a half-size square instead of a full one).
    pieces = []                  # (chunk, lo, hi) in floats, lo/hi % 25 == 0
    for i in range(n):
        sz = schedule[i]
        if i < 2 and sz >= 2 * GROUP:
            half = (sz // 2 // GROUP) * GROUP
            pieces.append((i, 0, half))
            pieces.append((i, half, sz))
        else:
            pieces.append((i, 0, sz))
    last_piece = {}              # chunk -> index of its last piece
    for pidx, (c, _, _) in enumerate(pieces):
        last_piece[c] = pidx
    r_of = last_piece            # reduce ops mirror pieces 1:1

    gs_all = nc.alloc_sbuf_tensor("gs_all", [P, total_g], f32).ap()
    pr = nc.alloc_sbuf_tensor("pr", [P, n_segs], f32).ap()
    dm = nc.alloc_sbuf_tensor("dm_scratch", [1, 1], f32).ap()
    ones = nc.const_aps.aps[(f32, 1.0)]   # preamble-initialized [128, 1]

    dma_sems = [nc.alloc_semaphore(f"dma_sem{b}") for b in range(Bi)]
    out_sem = nc.alloc_semaphore("out_sem")
    sq_sem = nc.alloc_semaphore("sq_sem")       # ACT square piece done
    red_sem = nc.alloc_semaphore("red_sem")     # DVE reduce piece done
    sqrt_sem = nc.alloc_semaphore("sqrt_sem")   # ACT segment sqrts done
    tt_sem = nc.alloc_semaphore("tt_sem")       # DVE pair-fold done (RAW
    # guard: DVE pipelines instructions, so the reduce reading the fold's
    # output needs an explicit completion sem even on the same engine)

    # column strips per chunk: striped chunks get several DMA instructions
    # alternating SP (even strips) / ACT (odd strips); others one on SP.
    # arrive[c] = dma_sems[slot] value once chunk c's data is complete.
    def strips_of(c):
        k = stripe_chunks.get(c, 1)
        sz = schedule[c]
        per = (sz // k // GROUP) * GROUP
        cuts = [i * per for i in range(k)] + [sz]
        return [(cuts[i], cuts[i + 1]) for i in range(k)]

    # gpsimd-issued chunks use their own sems (a sem driven by SWDGE can't
    # also be updated by HWDGE) and don't advance the slot's HWDGE counter
    gp_sems = {c: nc.alloc_semaphore(f"gp_dma_sem{c}")
               for c in sorted(gp_dma_chunks)}
    # odd chunks issued from ACT's queue. A slot's chunks (c, c+Bi, ...)
    # share parity when Bi is even, so each dma_sems[slot] stays
    # single-queue-FIFO (except striped chunks, where thresholds are
    # cumulative sums and order doesn't matter).
    if act_alternate:
        assert Bi % 2 == 0
        act_chunks = {c for c in range(4, n)
                      if c % 2 and c not in gp_dma_chunks
                      and c not in stripe_chunks}
    else:
        act_chunks = set()
    arrive = {}
    slot_total = [0] * Bi
    for c in range(n):
        if c in gp_dma_chunks:
            arrive[c] = 16
            continue
        slot_total[c % Bi] += 16 * len(strips_of(c))
        arrive[c] = slot_total[c % Bi]

    def emit_chunk_dma(eng, c, parity):
        sem = gp_sems.get(c, dma_sems[c % Bi])
        for s, (lo, hi) in enumerate(strips_of(c)):
            if s % 2 == parity:
                eng.dma_start(
                    t_in[c % Bi][:, lo:hi],
                    x[:, offs[c] + lo:offs[c] + hi],
                ).then_inc(sem, 16)

    def emit_sp(sp):
        for i in range(n):
            if i in gp_dma_chunks or i in act_chunks:
                continue
            if i >= Bi:
                # input slot free once ACT consumed the chunk B_in back
                sp.wait_ge(sq_sem, r_of[i - Bi] + 1)
            emit_chunk_dma(sp, i, 0)
        # final output: pr's accumulator writes retire (sqrt_sem counts the
        # per-segment ACTIVATION_READ_ACCUMULATORs) -> DMA pr out.
        # No completion wait: the NEFF's end barrier + ~7us semaphore-restore
        # postamble + runtime queue quiesce cover the 2.5KB transfer's
        # landing, and nothing ever waits on out_sem (a stale nonzero value
        # across executions is harmless). Dropping the wait moves every
        # engine's barrier arrival ~1.9us earlier.
        sp.wait_ge(sqrt_sem, n_segs)
        sp.dma_start(out, pr).then_inc(out_sem, 16)

    def emit_act(act):
        # odd strips of the striped chunks on ACT's own HWDGE queue: two
        # descriptor generators fill the SDMA rings in parallel at the start
        for c in sorted(stripe_chunks):
            emit_chunk_dma(act, c, 1)
        # ACT-owned chunks on fresh slots go out up front; later ones are
        # issued right after the square of their slot's previous occupant
        for c in sorted(act_chunks):
            if c < Bi:
                emit_chunk_dma(act, c, 0)
        # table prefetch: first activation is a Sqrt, so the one table set
        # loaded (sqrt_and_others) also covers Square -> no mid-kernel load
        act.activation(dm, ones[0:1, :], Act.Sqrt)

        emitted = 0

        def emit_seg(s):
            end_chunk, glo, ghi = segs[s]
            act.wait_ge(red_sem, r_of[end_chunk - 1] + 1)
            act.activation(gs_all[:, glo:ghi], gs_all[:, glo:ghi], Act.Sqrt,
                           accum_out=pr[:, s:s + 1]).then_inc(sqrt_sem, 1)

        prev_chunk = -1
        for pidx, (c, lo, hi) in enumerate(pieces):
            if c != prev_chunk:
                while emitted < n_segs and seg_emit_before.get(emitted) == c:
                    emit_seg(emitted)
                    emitted += 1
                t = c + Bi - 2
                if c >= 2 and t in act_chunks and t < n:
                    # issue the ACT-owned chunk whose slot was freed by the
                    # square of chunk c-2 (that sq_sem fired a full square
                    # ago, so this wait never stalls; it exists because a
                    # dma_start only enqueues and the engine pipelines)
                    act.wait_ge(sq_sem, r_of[c - 2] + 1)
                    emit_chunk_dma(act, t, 0)
                act.wait_ge(gp_sems.get(c, dma_sems[c % Bi]), arrive[c])
                if c >= Bs:
                    # sq slot free once DVE reduced the chunk B_sq back
                    act.wait_ge(red_sem, r_of[c - Bs] + 1)
                prev_chunk = c
            act.activation(t_sq[c % Bs][:, lo:hi], t_in[c % Bi][:, lo:hi],
                           Act.Square).then_inc(sq_sem, 1)
        for s in range(emitted, n_segs):
            emit_seg(s)

    def emit_dve(dve):
        add = mybir.AluOpType.add
        n_tt = 0
        for pidx, (c, lo, hi) in enumerate(pieces):
            dve.wait_ge(sq_sem, pidx + 1)
            base = (c % Bs) * max_sz
            g = (hi - lo) // GROUP
            sqA = sq_ring[:, base + lo:base + hi].rearrange(
                "p (g k) -> p g k", k=GROUP)
            gs_out = gs_all[:, goffs[c] + lo // GROUP:goffs[c] + hi // GROUP]
            if g < 50:
                # small piece: the plain reduce's fixed cost beats 2 instrs
                dve.reduce_sum(gs_out, sqA, axis=mybir.AxisListType.X
                               ).then_inc(red_sem, 1)
                continue
            # pair-fold in place: tensor_tensor runs 2 elem/cycle for bf16
            # while tensor_reduce is capped at 1 elem/cycle for all dtypes,
            # so folding 25 -> 13 before the reduce cuts DVE ~20%/chunk.
            # cols 13..24 += cols 0..11, then reduce contiguous cols 12..24.
            dve.tensor_tensor(sqA[:, :, 13:25], sqA[:, :, 13:25],
                              sqA[:, :, 0:12], add).then_inc(tt_sem, 1)
            n_tt += 1
            dve.wait_ge(tt_sem, n_tt)
            dve.reduce_sum(gs_out, sqA[:, :, 12:25],
                           axis=mybir.AxisListType.X).then_inc(red_sem, 1)

    def emit_gp(gp):
        for c in sorted(gp_dma_chunks):
            emit_chunk_dma(gp, c, 0)

    emit_sp(nc.sync)
    emit_act(nc.scalar)
    emit_dve(nc.vector)
    if gp_dma_chunks:
        emit_gp(nc.gpsimd)

    nc.compile()
    return nc


def kernel(weight, c_omega):
    global _compiled, LAST_RESULTS
    from concourse.bass_utils import run_bass_kernel_spmd

    if _compiled is None:
        _compiled = build()
    nc = _compiled

    w = np.asarray(weight)
    if w.dtype != np.float32:
        w = w.astype(np.float32)
    w = np.ascontiguousarray(w)
    flat = w.reshape(-1)
    per_core = flat.size // N_CORES
    in_maps = [
        {"x": flat[c * per_core:(c + 1) * per_core].reshape(P, F_PER_PART)}
        for c in range(N_CORES)
    ]
    LAST_RESULTS = run_bass_kernel_spmd(nc, in_maps,
                                        core_ids=list(range(N_CORES)))
    total = 0.0
    for r in LAST_RESULTS.results:
        total += float(r["out"].astype(np.float64).sum())
    loss = total / N_ROWS * (C_OMEGA * float(c_omega))
    return np.float32(loss)


def selftest_sim(f_per_part=625, schedule=(250, 250, 75, 25, 25),
                 in_bufs=3, sq_bufs=3, seed=0, **kw):
    """CoreSim check on a scaled-down instance; returns max rel err."""
    from concourse.bass_interp import CoreSim

    nc = build(f_per_part=f_per_part, schedule=list(schedule),
               in_bufs=in_bufs, sq_bufs=sq_bufs, **kw)
    rng = np.random.default_rng(seed)
    xv = rng.standard_normal((P, f_per_part)).astype(np.float32)
    sim = CoreSim(nc)
    sim.tensor("x")[:] = xv
    sim.simulate()
    got = float(np.array(sim.tensor("out")).astype(np.float64).sum())
    g = xv.reshape(P, f_per_part // GROUP, GROUP)
    want = float(np.sqrt((g.astype(np.float64) ** 2).sum(-1)).sum())
    return abs(got - want) / abs(want)
